# revision 8
# baseline (speedup 1.0000x reference)
# Circular convolution along channels == matmul with a circulant matrix:
#   y[r, n] = sum_k x[r, k] * W[(n - k) mod 2048],  W = W_first_col * W_second_col
# Shard rows (8*64*64 = 32768) across 8 NeuronCores; replicate the 2048x1536
# circulant matrix. Per core: [4096, 2048] @ [2048, 1536] fp16 matmul, fp32 out
# (fp16 runs at the same 1 cycle/row PE rate as bf16 but has 10 mantissa bits).
import numpy as np

IN_DIM = 2048
OUT_DIM = 1536
N_CORES = 8
ROWS = 8 * 64 * 64            # 32768
RPC = ROWS // N_CORES         # 4096 rows per core

P = 128                       # partitions
K_TILES = IN_DIM // P         # 16
N_TILE = 512                  # PSUM bank = 512 fp32
N_CHUNKS = OUT_DIM // N_TILE  # 3
ROW_TILE = 512                # rows per streamed x tile
N_ROW_TILES = RPC // ROW_TILE     # 8
RB_PER_TILE = ROW_TILE // P       # 4 row-blocks per x tile

_cache = {}


def _emit_body(nc, xpool, opool, pspool, wt, xT, y):
    import concourse.mybir as mybir

    for rt in range(N_ROW_TILES):
        xts = []
        for kt in range(K_TILES):
            xt_t = xpool.tile([P, ROW_TILE], mybir.dt.float16,
                              name=f"x{kt}_{rt}", tag=f"x{kt}")
            nc.sync.dma_start(
                xt_t[:],
                xT[kt * P:(kt + 1) * P, rt * ROW_TILE:(rt + 1) * ROW_TILE],
            )
            xts.append(xt_t)

        for rb in range(RB_PER_TILE):
            ps = pspool.tile([P, N_CHUNKS, N_TILE], mybir.dt.float32,
                             name=f"ps_{rt}_{rb}", tag="ps")
            for kt in range(K_TILES):
                lhsT = xts[kt][:, rb * P:(rb + 1) * P]
                for n in range(N_CHUNKS):
                    nc.tensor.matmul(
                        ps[:, n, :],
                        lhsT,
                        wt[kt][:, n * N_TILE:(n + 1) * N_TILE],
                        start=(kt == 0),
                        stop=(kt == K_TILES - 1),
                    )
            ot = opool.tile([P, OUT_DIM], mybir.dt.float32,
                            name=f"o_{rt}_{rb}", tag="ot")
            for n in range(N_CHUNKS):
                nc.vector.tensor_copy(ot[:, n * N_TILE:(n + 1) * N_TILE],
                                      ps[:, n, :])
            row0 = rt * ROW_TILE + rb * P
            nc.sync.dma_start(y[row0:row0 + P, :], ot[:])


def _build(repeat=1):
    import contextlib

    import concourse.bass as bass
    import concourse.mybir as mybir
    import concourse.tile as tile
    from concourse import bacc

    nc = bacc.Bacc(
        "TRN2",
        target_bir_lowering=False,
        debug=False,
        enable_asserts=False,
        num_devices=N_CORES,
    )
    xT = nc.dram_tensor("xT", (IN_DIM, RPC), mybir.dt.float16, kind="ExternalInput")
    mm = nc.dram_tensor("mm", (IN_DIM, OUT_DIM), mybir.dt.float16, kind="ExternalInput")
    y = nc.dram_tensor("y", (RPC, OUT_DIM), mybir.dt.float32, kind="ExternalOutput")

    with tile.TileContext(nc) as tc:
        with (
            tc.tile_pool(name="w", bufs=1) as wpool,
            tc.tile_pool(name="x", bufs=3) as xpool,
            tc.tile_pool(name="o", bufs=3) as opool,
            tc.tile_pool(name="ps", bufs=2, space=bass.MemorySpace.PSUM) as pspool,
        ):
            # resident circulant weights: 16 k-tiles of [128, 1536] fp16 (6 MB).
            # Split the preload across the gpsimd/scalar DMA rings so it
            # streams concurrently with the x loads on the sync ring.
            wt = []
            for kt in range(K_TILES):
                w = wpool.tile([P, OUT_DIM], mybir.dt.float16,
                               name=f"w{kt}", tag=f"w{kt}")
                e = nc.gpsimd if kt % 2 == 0 else nc.scalar
                e.dma_start(w[:], mm[kt * P:(kt + 1) * P, :])
                wt.append(w)

            if repeat > 1:
                with tc.For_i(0, repeat, 1):
                    _emit_body(nc, xpool, opool, pspool, wt, xT, y)
            else:
                _emit_body(nc, xpool, opool, pspool, wt, xT, y)

    nc.compile()
    return nc


def kernel(x: np.ndarray, W_first_col: np.ndarray, W_second_col: np.ndarray) -> np.ndarray:
    from concourse import bass_utils

    W = (np.asarray(W_first_col, np.float32)
         * np.asarray(W_second_col, np.float32))[:IN_DIM]
    # circulant: mmat[k, n] = W[(n - k) mod IN_DIM]
    idx = (np.arange(OUT_DIM)[None, :] - np.arange(IN_DIM)[:, None]) % IN_DIM
    mmat = np.ascontiguousarray(W[idx]).astype(np.float16)

    xf = np.asarray(x, np.float32).reshape(ROWS, IN_DIM)
    in_maps = []
    for c in range(N_CORES):
        shard = xf[c * RPC:(c + 1) * RPC].astype(np.float16)
        xTc = np.ascontiguousarray(shard.T)  # [IN_DIM, RPC]
        in_maps.append({"xT": xTc, "mm": mmat})

    if "nc" not in _cache:
        _cache["nc"] = _build()
    res = bass_utils.run_bass_kernel_spmd(
        _cache["nc"], in_maps, core_ids=list(range(N_CORES))
    )
    out = np.concatenate([r["y"] for r in res.results], axis=0)
    return out.reshape(8, 64, 64, OUT_DIM)
